# revision 1
# baseline (speedup 1.0000x reference)
"""Trainium2 Bass kernel for causal multi-head attention block.

Reference computation (B=4, S=2048, D=1024, H=16, HD=64, fp32):
    qkv = x @ Wqkv + bqkv; split q,k,v; per-head scaled scores;
    causal mask filled with -0.0001 (leaky, NOT -inf); softmax over all
    2048 keys; out = P @ V; out = out @ Wo + bo.

Sharding: 8 cores, core = (batch b = i//2, parity p = i%2). Each core
computes 1024 queries of its batch: query tiles t = 2j+p (j=0..3) of
256 queries. Causal block structure (512-key blocks per 256-query tile)
is then identical on every core: qtile j needs key blocks 0..j -> one
SPMD program, zero cross-core communication.

The leaky mask is handled exactly:
  - computed blocks: E' = exp(S)*M + (1-M)*w  with w = exp(-1e-4)
  - skipped key blocks (all masked): contribute w*SufV[j] to the
    numerator (suffix sums of V at 512-block granularity) and
    w*n_skip to the denominator Z.
Z is produced inside the PV matmul via a 65th all-ones V column.
Matmuls run as float32r (full-rate fp32 streaming); Q/K are stored
bf16 so the dh=64 score matmuls stream at full fetch rate.
"""

import math
from contextlib import ExitStack

import numpy as np

import concourse.bass as bass
import concourse.mybir as mybir
import concourse.tile as tile
from concourse import bacc

F32 = mybir.dt.float32
F32R = mybir.dt.float32r
BF16 = mybir.dt.bfloat16
AF = mybir.ActivationFunctionType
ALU = mybir.AluOpType
AX = mybir.AxisListType

B, S, D, H, HD = 4, 2048, 1024, 16, 64
QL, QT, KB, NJ = 1024, 256, 512, 4    # queries/core, qtile, key block, n qtiles
NCH = D // 128                         # contraction chunks
PAIRS = H // 2
W_MASK = math.exp(-1e-4)


def _r(ap):
    return ap


def build_program():
    nc = bacc.Bacc(
        "TRN2",
        target_bir_lowering=False,
        debug=False,
        num_devices=8,
    )
    xT = nc.declare_dram_parameter("xT", [D, S], F32R, isOutput=False)
    xqT = nc.declare_dram_parameter("xqT", [D, QL], F32R, isOutput=False)
    wqkv = nc.declare_dram_parameter("wqkv", [D, 3 * D], F32R, isOutput=False)
    wo = nc.declare_dram_parameter("wo", [D, D], F32R, isOutput=False)
    b2h = nc.declare_dram_parameter("b2h", [128, 24], F32, isOutput=False)
    brow = nc.declare_dram_parameter("brow", [1, 3 * D], F32R, isOutput=False)
    bv512 = nc.declare_dram_parameter("bv512", [128, 8], F32, isOutput=False)
    bocol = nc.declare_dram_parameter("bocol", [128, 8], F32, isOutput=False)
    mmul = nc.declare_dram_parameter("mmul", [128, 4 * QT], F32R, isOutput=False)
    madd = nc.declare_dram_parameter("madd", [128, 4 * QT], F32R, isOutput=False)
    onesd = nc.declare_dram_parameter("onesd", [128, 128], F32R, isOutput=False)
    outT = nc.declare_dram_parameter("outT", [D, QL], F32, isOutput=True)

    with tile.TileContext(nc) as tc, ExitStack() as ctx, \
         nc.allow_low_precision(reason="float32r matmul inputs are fp32 bits"):
        consts = ctx.enter_context(tc.tile_pool(name="consts", bufs=1))
        b2h_sb = consts.tile([128, 24], F32)
        nc.sync.dma_start(out=b2h_sb, in_=b2h[:])
        brow_sb = consts.tile([1, D], F32R)
        nc.sync.dma_start(out=brow_sb, in_=brow[0:1, 2 * D:3 * D])
        bv512_sb = consts.tile([128, 8], F32)
        nc.sync.dma_start(out=bv512_sb, in_=bv512[:])
        bocol_sb = consts.tile([128, 8], F32)
        nc.sync.dma_start(out=bocol_sb, in_=bocol[:])
        mmul_sb = consts.tile([128, 4 * QT], F32R)
        nc.sync.dma_start(out=mmul_sb, in_=mmul[:])
        madd_sb = consts.tile([128, 4 * QT], F32R)
        nc.sync.dma_start(out=madd_sb, in_=madd[:])
        ones_sb = consts.tile([1, 128], F32R)
        nc.sync.dma_start(out=ones_sb, in_=onesd[0:1, :])

        with ExitStack() as ctx2:
            xt_pool = ctx2.enter_context(tc.tile_pool(name="xt", bufs=1))
            xT_sb = xt_pool.tile([128, NCH, S], F32R)
            for c in range(NCH):
                nc.sync.dma_start(out=xT_sb[:, c, :], in_=xT[128 * c:128 * (c + 1), :])

            # per-512-block column sums of xT (for V block-sum corrections)
            xsum_sb = consts.tile([128, NCH, 4], F32R)
            for c in range(NCH):
                nc.vector.tensor_reduce(
                    out=xsum_sb[:, c, :],
                    in_=xT_sb[:, c, :].rearrange("p (b t) -> p b t", b=4),
                    axis=AX.X, op=ALU.add,
                )

            # ---------- Q projection, all head pairs up front ----------
            qt_pool = ctx2.enter_context(tc.tile_pool(name="qt", bufs=1))
            QT_all = qt_pool.tile([128, PAIRS, QL], BF16)
            with tc.tile_pool(name="xq", bufs=1) as xq_pool, \
                 tc.tile_pool(name="wq", bufs=2) as wq_pool, \
                 tc.tile_pool(name="qps", bufs=2, space="PSUM") as qps_pool:
                xqT_sb = xq_pool.tile([128, NCH, QL], F32R)
                for c in range(NCH):
                    nc.sync.dma_start(out=xqT_sb[:, c, :], in_=xqT[128 * c:128 * (c + 1), :])
                for pr in range(PAIRS):
                    wq_sb = wq_pool.tile([128, NCH, 128], F32R)
                    nc.sync.dma_start(
                        out=wq_sb,
                        in_=wqkv[:, 128 * pr:128 * (pr + 1)].rearrange("(c p) m -> p c m", p=128),
                    )
                    for g2 in range(2):
                        ps = qps_pool.tile([128, 512], F32)
                        for c in range(NCH):
                            nc.tensor.matmul(
                                out=ps, lhsT=_r(wq_sb[:, c, :]),
                                rhs=_r(xqT_sb[:, c, 512 * g2:512 * (g2 + 1)]),
                                start=(c == 0), stop=(c == NCH - 1),
                            )
                        # QT = (x@Wq)*0.125 + bq/8  (bias columns pre-divided on host)
                        nc.vector.tensor_scalar(
                            out=QT_all[:, pr, 512 * g2:512 * (g2 + 1)], in0=ps,
                            scalar1=0.125, scalar2=b2h_sb[:, pr:pr + 1],
                            op0=ALU.mult, op1=ALU.add,
                        )

            # ---------- main loop: 4 groups of 4 heads ----------
            vpool = ctx2.enter_context(tc.tile_pool(name="vsb", bufs=2))
            kt_pool = ctx2.enter_context(tc.tile_pool(name="kt", bufs=2))
            odram = ctx2.enter_context(tc.tile_pool(name="odram", bufs=1, space="DRAM"))
            O_dr = odram.tile([D, QL], F32R)       # [h*d, q] transposed head outputs

            with tc.tile_pool(name="wv", bufs=2) as wv_pool, \
                 tc.tile_pool(name="wk", bufs=2) as wk_pool, \
                 tc.tile_pool(name="pps", bufs=2, space="PSUM") as pps_pool, \
                 tc.tile_pool(name="sps", bufs=2, space="PSUM") as sps_pool, \
                 tc.tile_pool(name="ops", bufs=2, space="PSUM") as ops_pool, \
                 tc.tile_pool(name="esb", bufs=4) as e_pool, \
                 tc.tile_pool(name="bs", bufs=2) as bs_pool, \
                 tc.tile_pool(name="osb", bufs=4) as osb_pool, \
                 tc.tile_pool(name="misc", bufs=4) as misc_pool:

                for g in range(4):
                    # V projection for this group's 4 heads (token-major, 65th ones col)
                    wv_sb = wv_pool.tile([128, NCH, 256], F32R)
                    nc.sync.dma_start(
                        out=wv_sb,
                        in_=wqkv[:, 2 * D + 256 * g:2 * D + 256 * (g + 1)].rearrange("(c p) m -> p c m", p=128),
                    )
                    V_sb = vpool.tile([128, 16, 4, 65], F32R)
                    nc.sync.dma_start(
                        out=V_sb[:, :, :, 64],
                        in_=onesd[:, 0:64].rearrange("p (t g) -> p t g", t=16),
                    )
                    for t in range(16):
                        ps = pps_pool.tile([128, 256], F32, tag="pps")
                        for c in range(NCH):
                            nc.tensor.matmul(
                                out=ps, lhsT=_r(xT_sb[:, c, 128 * t:128 * (t + 1)]),
                                rhs=_r(wv_sb[:, c, :]),
                                start=(c == 0), stop=False,
                            )
                        nc.tensor.matmul(
                            out=ps, lhsT=_r(ones_sb),
                            rhs=_r(brow_sb[:, 256 * g:256 * (g + 1)]),
                            start=False, stop=True,
                        )
                        nc.vector.tensor_copy(
                            out=V_sb[:, t, :, 0:64],
                            in_=ps.rearrange("p (h d) -> p h d", h=4),
                        )

                    for lp in range(2):
                        pr = 2 * g + lp
                        # W-scaled per-block V column sums -> suffix sums
                        psb = pps_pool.tile([128, 4], F32, tag="pps")
                        for c in range(NCH):
                            nc.tensor.matmul(
                                out=psb, lhsT=_r(wv_sb[:, c, 128 * lp:128 * (lp + 1)]),
                                rhs=_r(xsum_sb[:, c, :]),
                                start=(c == 0), stop=(c == NCH - 1),
                            )
                        bs_sb = bs_pool.tile([128, 4], F32, tag="bs")
                        nc.vector.tensor_scalar(
                            out=bs_sb, in0=psb, scalar1=W_MASK,
                            scalar2=bv512_sb[:, pr:pr + 1], op0=ALU.mult, op1=ALU.add,
                        )
                        suf_sb = bs_pool.tile([128, 4], F32, tag="suf")
                        nc.vector.memset(suf_sb[:, 3:4], 0.0)
                        nc.vector.tensor_copy(out=suf_sb[:, 2:3], in_=bs_sb[:, 3:4])
                        nc.vector.tensor_add(out=suf_sb[:, 1:2], in0=bs_sb[:, 2:3], in1=suf_sb[:, 2:3])
                        nc.vector.tensor_add(out=suf_sb[:, 0:1], in0=bs_sb[:, 1:2], in1=suf_sb[:, 1:2])

                        # K projection for this pair (d-major)
                        wk_sb = wk_pool.tile([128, NCH, 128], F32R)
                        nc.sync.dma_start(
                            out=wk_sb,
                            in_=wqkv[:, D + 128 * pr:D + 128 * (pr + 1)].rearrange("(c p) m -> p c m", p=128),
                        )
                        KT_sb = kt_pool.tile([128, S], BF16)
                        for kg in range(4):
                            ps = pps_pool.tile([128, 512], F32, tag="pps")
                            for c in range(NCH):
                                nc.tensor.matmul(
                                    out=ps, lhsT=_r(wk_sb[:, c, :]),
                                    rhs=_r(xT_sb[:, c, 512 * kg:512 * (kg + 1)]),
                                    start=(c == 0), stop=(c == NCH - 1),
                                )
                            nc.vector.tensor_scalar_add(
                                out=KT_sb[:, 512 * kg:512 * (kg + 1)], in0=ps,
                                scalar1=b2h_sb[:, 8 + pr:9 + pr],
                            )

                        # attention, 2 heads interleaved per qtile to keep PE dense
                        for j in range(NJ):
                            for hl in range(2):
                                ghl = 2 * lp + hl
                                hsl = slice(64 * hl, 64 * (hl + 1))
                                # po cols 0:256 = PV accum + Z row; cols 256:512 = Z broadcast
                                po = ops_pool.tile([65, 512], F32, tag="ops")
                                for kb in range(j + 1):
                                    diag = kb == j
                                    pss = sps_pool.tile([128, 4, 256], F32)
                                    for s2 in range(4):
                                        nc.tensor.matmul(
                                            out=pss[:, s2, :],
                                            lhsT=_r(KT_sb[hsl, 512 * kb + 128 * s2:512 * kb + 128 * (s2 + 1)]),
                                            rhs=_r(QT_all[hsl, pr, 256 * j:256 * (j + 1)]),
                                            start=True, stop=True,
                                        )
                                    e_sb = e_pool.tile([128, 4, 256], F32R)
                                    nc.scalar.activation(out=e_sb, in_=pss, func=AF.Exp)
                                    if diag:
                                        ef = e_sb[:].rearrange("p a b -> p (a b)")
                                        nc.vector.tensor_mul(out=ef, in0=ef, in1=mmul_sb[:])
                                        nc.vector.tensor_add(out=ef, in0=ef, in1=madd_sb[:])
                                    for s2 in range(4):
                                        nc.tensor.matmul(
                                            out=po[:, 0:256],
                                            lhsT=_r(V_sb[:, 4 * kb + s2, ghl, :]),
                                            rhs=_r(e_sb[:, s2, :]),
                                            start=(kb == 0 and s2 == 0),
                                            stop=(kb == j and s2 == 3),
                                            skip_group_check=True,
                                        )
                                # epilogue: Z, broadcast, numerator correction, divide
                                nskip = S - KB * (j + 1)
                                zf = misc_pool.tile([1, 256], F32, tag="zf")
                                nc.vector.tensor_scalar_add(out=zf, in0=po[64:65, 0:256], scalar1=W_MASK * nskip)
                                zi = misc_pool.tile([1, 256], F32, tag="zi")
                                nc.vector.reciprocal_approx_fast(out=zi, in_=zf)
                                zr = misc_pool.tile([1, 256], F32R, tag="zr")
                                nc.vector.tensor_copy(out=zr, in_=zi)
                                nc.tensor.matmul(out=po[0:64, 256:512], lhsT=_r(ones_sb[:, 0:64]), rhs=_r(zr), start=True, stop=True)
                                nm = misc_pool.tile([64, 256], F32, tag="nm")
                                nc.vector.tensor_scalar_add(
                                    out=nm, in0=po[0:64, 0:256], scalar1=suf_sb[hsl, j:j + 1],
                                )
                                ot = osb_pool.tile([64, 256], F32R, tag="ot")
                                nc.vector.tensor_mul(out=ot, in0=nm, in1=po[0:64, 256:512])
                                nc.sync.dma_start(
                                    out=O_dr[128 * pr + 64 * hl:128 * pr + 64 * (hl + 1), 256 * j:256 * (j + 1)],
                                    in_=ot,
                                )

        # ---------- output projection ----------
        with tc.tile_pool(name="wosb", bufs=1) as wo_pool, \
             tc.tile_pool(name="ochunk", bufs=2) as oc_pool, \
             tc.tile_pool(name="fps", bufs=2, space="PSUM") as fps_pool, \
             tc.tile_pool(name="fout", bufs=3) as fo_pool:
            wo_sb = wo_pool.tile([128, NCH, 8, 128], F32R)
            nc.sync.dma_start(
                out=wo_sb,
                in_=wo[:].rearrange("(c p) (t m) -> p c t m", p=128, m=128),
            )
            for j in range(NJ):
                oj = oc_pool.tile([128, NCH, 256], F32R)
                nc.sync.dma_start(
                    out=oj,
                    in_=O_dr[:, 256 * j:256 * (j + 1)].rearrange("(c p) q -> p c q", p=128),
                )
                for dt_ in range(8):
                    ps = fps_pool.tile([128, 256], F32)
                    for c in range(NCH):
                        nc.tensor.matmul(
                            out=ps, lhsT=_r(wo_sb[:, c, dt_, :]), rhs=_r(oj[:, c, :]),
                            start=(c == 0), stop=(c == NCH - 1),
                        )
                    fo = fo_pool.tile([128, 256], F32)
                    nc.vector.tensor_scalar_add(out=fo, in0=ps, scalar1=bocol_sb[:, dt_:dt_ + 1])
                    nc.sync.dma_start(
                        out=outT[128 * dt_:128 * (dt_ + 1), 256 * j:256 * (j + 1)],
                        in_=fo,
                    )
    nc.compile()
    return nc


def qrows_for(p):
    return np.concatenate(
        [np.arange(QT * (2 * j + p), QT * (2 * j + p) + QT) for j in range(NJ)]
    )


def host_in_maps(x, Wqkv, bqkv, Wo, bo):
    x = np.ascontiguousarray(np.asarray(x, np.float32))
    Wqkv = np.ascontiguousarray(np.asarray(Wqkv, np.float32))
    bqkv = np.asarray(bqkv, np.float32)
    Wo = np.ascontiguousarray(np.asarray(Wo, np.float32))
    bo = np.asarray(bo, np.float32)

    b2h = np.ascontiguousarray(bqkv.reshape(24, 128).T)
    b2h[:, 0:8] /= 8.0
    brow = bqkv.reshape(1, 3 * D)
    bv512 = np.ascontiguousarray((W_MASK * 512.0 * bqkv[2 * D:].reshape(8, 128)).T)
    bocol = np.ascontiguousarray(bo.reshape(8, 128).T)
    onesd = np.ones((128, 128), np.float32)

    kap = np.arange(128)[:, None]
    r = np.arange(QT)[None, :]
    masks = {}
    for p in range(2):
        mm = np.zeros((128, 4, QT), np.float32)
        for s in range(4):
            mm[:, s, :] = (128 * s + kap <= QT * p + r)
        mm2 = np.ascontiguousarray(mm.reshape(128, 4 * QT))
        masks[p] = (mm2, np.ascontiguousarray((1.0 - mm2) * W_MASK))

    in_maps = []
    for core in range(8):
        b, p = core // 2, core % 2
        mma, mada = masks[p]
        in_maps.append({
            "xT": np.ascontiguousarray(x[b].T),
            "xqT": np.ascontiguousarray(x[b][qrows_for(p)].T),
            "wqkv": Wqkv,
            "wo": Wo,
            "b2h": b2h,
            "brow": brow,
            "bv512": bv512,
            "bocol": bocol,
            "onesd": onesd,
            "mmul": mma,
            "madd": mada,
        })
    return in_maps


_CACHED = {}


def get_program():
    if "nc" not in _CACHED:
        _CACHED["nc"] = build_program()
    return _CACHED["nc"]


def kernel(x, Wqkv, bqkv, Wo, bo):
    from concourse.bass_utils import run_bass_kernel_spmd

    nc = get_program()
    in_maps = host_in_maps(x, Wqkv, bqkv, Wo, bo)
    res = run_bass_kernel_spmd(nc, in_maps, core_ids=list(range(8)))
    out = np.zeros((B, S, D), np.float32)
    for core in range(8):
        b, p = core // 2, core % 2
        out[b, qrows_for(p), :] = res.results[core]["outT"].T
    return out



# revision 24
# speedup vs baseline: 1.3676x; 1.3676x over previous
"""Trainium2 Bass kernel for causal multi-head attention block (v2).

Reference computation (B=4, S=2048, D=1024, H=16, HD=64, fp32):
    qkv = x @ Wqkv + bqkv; split q,k,v; per-head scaled scores;
    causal mask filled with -0.0001 (leaky, NOT -inf); softmax over all
    2048 keys; out = P @ V; out = out @ Wo + bo.

Sharding: 8 cores, core = (batch b = i//2, parity p = i%2). Each core
computes 1024 queries of its batch: query tiles t = 2j+p (j=0..3) of
256 queries -> identical SPMD program, zero cross-core communication.

v2 design vs v1 baseline:
  - all inputs bf16 host-side (half DMA, FWL weight loads, 2x DVE).
  - score matmuls packed 2 heads/slot via PE row tiling (tile_position
    (0,0)/(64,0) auto-derived from 64-partition slices -> concurrent).
  - leaky diag mask via one copy_predicated (w at masked positions).
  - V bias folded into bo' = bo + bv @ Wo on host (exact).
  - per-qtile suffix correction (skipped blocks) and w*nskip Z term
    injected via a single K=1 rank-1 matmul into the PV PSUM group.
  - 1/8 score scale folded into the exp activation's scale field.
  - all weights resident in SBUF up front; K-proj of pair pr+1 emitted
    inside pair pr's attention loop as PE filler (keeps HAM warm).
  - attention output kept in SBUF (no DRAM roundtrip).
"""

import math
from contextlib import ExitStack

import numpy as np

import concourse.bass as bass
import concourse.mybir as mybir
import concourse.tile as tile
from concourse import bacc

F32 = mybir.dt.float32
F32R = mybir.dt.float32r
BF16 = mybir.dt.bfloat16
AF = mybir.ActivationFunctionType
ALU = mybir.AluOpType
AX = mybir.AxisListType

B, S, D, H, HD = 4, 2048, 1024, 16, 64
QL, QT, KB, NJ = 1024, 256, 512, 4    # queries/core, qtile, key block, n qtiles
NCH = D // 128                         # contraction chunks
PAIRS = H // 2
W_MASK = math.exp(-1e-4)


def build_program():
    nc = bacc.Bacc(
        "TRN2",
        target_bir_lowering=False,
        debug=False,
        num_devices=8,
    )
    xT = nc.declare_dram_parameter("xT", [D, S], BF16, isOutput=False)
    xqT = nc.declare_dram_parameter("xqT", [D, QL], BF16, isOutput=False)
    wq = nc.declare_dram_parameter("wq", [D, D], BF16, isOutput=False)
    wk = nc.declare_dram_parameter("wk", [D, D], BF16, isOutput=False)
    wv = nc.declare_dram_parameter("wv", [D, D], BF16, isOutput=False)
    wo = nc.declare_dram_parameter("wo", [D, D], BF16, isOutput=False)
    bq2 = nc.declare_dram_parameter("bq2", [128, 8], F32, isOutput=False)
    bk2 = nc.declare_dram_parameter("bk2", [128, 8], F32, isOutput=False)
    bo2 = nc.declare_dram_parameter("bo2", [128, 8], F32, isOutput=False)
    maskw = nc.declare_dram_parameter("maskw", [128, 4 * QT], BF16, isOutput=False)
    maskp = nc.declare_dram_parameter("maskp", [128, 4 * QT], mybir.dt.uint8, isOutput=False)
    onesd = nc.declare_dram_parameter("onesd", [1, 512], F32R, isOutput=False)
    outT = nc.declare_dram_parameter("outT", [D, QL], F32, isOutput=True)

    with tile.TileContext(nc) as tc, ExitStack() as ctx, \
         nc.allow_low_precision(reason="bf16 matmul inputs within rel-err budget"):
        consts = ctx.enter_context(tc.tile_pool(name="consts", bufs=1))
        bq_sb = consts.tile([128, 8], F32)
        nc.sync.dma_start(out=bq_sb, in_=bq2[:])
        bk_sb = consts.tile([128, 8], F32)
        nc.sync.dma_start(out=bk_sb, in_=bk2[:])
        bo_sb = consts.tile([128, 8], F32)
        nc.sync.dma_start(out=bo_sb, in_=bo2[:])
        maskw_sb = consts.tile([128, 4, QT], BF16)
        nc.sync.dma_start(out=maskw_sb, in_=maskw[:].rearrange("p (a b) -> p a b", a=4))
        maskp_sb = consts.tile([128, 4, QT], mybir.dt.uint8)
        nc.sync.dma_start(out=maskp_sb, in_=maskp[:].rearrange("p (a b) -> p a b", a=4))
        ones_sb = consts.tile([1, 512], F32R)
        nc.sync.dma_start(out=ones_sb, in_=onesd[:])

        # ---------- resident activations + weights ----------
        xt_pool = ctx.enter_context(tc.tile_pool(name="xt", bufs=1))
        xT_sb = xt_pool.tile([128, NCH, S], BF16)
        for c in range(NCH):
            nc.sync.dma_start(out=xT_sb[:, c, :], in_=xT[128 * c:128 * (c + 1), :])
        w_pool = ctx.enter_context(tc.tile_pool(name="wsb", bufs=1))
        wq_sb = w_pool.tile([128, NCH, D], BF16)
        nc.sync.dma_start(out=wq_sb, in_=wq[:].rearrange("(c p) m -> p c m", p=128))
        wk_sb = w_pool.tile([128, NCH, D], BF16)
        nc.sync.dma_start(out=wk_sb, in_=wk[:].rearrange("(c p) m -> p c m", p=128))
        wv_sb = w_pool.tile([128, NCH, D], BF16)
        nc.sync.dma_start(out=wv_sb, in_=wv[:].rearrange("(c p) m -> p c m", p=128))
        wo_sb = w_pool.tile([128, NCH, 8, 128], BF16)
        nc.sync.dma_start(
            out=wo_sb, in_=wo[:].rearrange("(c p) (t m) -> p c t m", p=128, m=128)
        )

        # per-512-block column sums of xT, pre-scaled by w (for V suffix sums)
        xsum_sb = consts.tile([128, NCH, 4], BF16)
        for c in range(NCH):
            nc.vector.tensor_reduce(
                out=xsum_sb[:, c, :],
                in_=xT_sb[:, c, :].rearrange("p (b t) -> p b t", b=4),
                axis=AX.X, op=ALU.add,
            )
        nc.gpsimd.tensor_scalar_mul(
            out=xsum_sb[:].rearrange("p c b -> p (c b)"),
            in0=xsum_sb[:].rearrange("p c b -> p (c b)"),
            scalar1=W_MASK,
        )

        qt_pool = ctx.enter_context(tc.tile_pool(name="qt", bufs=1))
        QT_all = qt_pool.tile([128, PAIRS, QL], BF16)
        v_pool = ctx.enter_context(tc.tile_pool(name="vsb", bufs=1))
        V_sb = v_pool.tile([128, 16, H, 65], BF16)
        nc.gpsimd.memset(V_sb[:, :, :, 64:65], 1.0)
        o_pool = ctx.enter_context(tc.tile_pool(name="osb", bufs=1))
        O_sb = o_pool.tile([128, NCH, QL], BF16)

        # ---------- Q projection (queries for this core only) ----------
        with tc.tile_pool(name="xq", bufs=1) as xq_pool, \
             tc.tile_pool(name="qps", bufs=2, space="PSUM") as qps_pool:
            xqT_sb = xq_pool.tile([128, NCH, QL], BF16)
            for c in range(NCH):
                nc.sync.dma_start(out=xqT_sb[:, c, :], in_=xqT[128 * c:128 * (c + 1), :])
            for pr in range(PAIRS):
                for g2 in range(2):
                    ps = qps_pool.tile([128, 512], F32)
                    for c in range(NCH):
                        nc.tensor.matmul(
                            out=ps, lhsT=wq_sb[:, c, 128 * pr:128 * (pr + 1)],
                            rhs=xqT_sb[:, c, 512 * g2:512 * (g2 + 1)],
                            start=(c == 0), stop=(c == NCH - 1),
                        )
                    nc.vector.tensor_scalar_add(
                        out=QT_all[:, pr, 512 * g2:512 * (g2 + 1)], in0=ps,
                        scalar1=bq_sb[:, pr:pr + 1],
                    )

        # ---------- V projection, all heads (token-major, 65th ones col) ----
        with tc.tile_pool(name="vps", bufs=2, space="PSUM") as vps_pool:
            for g in range(4):
                for t in range(16):
                    ps = vps_pool.tile([128, 256], F32)
                    for c in range(NCH):
                        nc.tensor.matmul(
                            out=ps, lhsT=xT_sb[:, c, 128 * t:128 * (t + 1)],
                            rhs=wv_sb[:, c, 256 * g:256 * (g + 1)],
                            start=(c == 0), stop=(c == NCH - 1),
                        )
                    nc.vector.tensor_copy(
                        out=V_sb[:, t, 4 * g:4 * (g + 1), 0:64],
                        in_=ps.rearrange("p (h d) -> p h d", h=4),
                    )

        # ---------- main loop over head pairs ----------
        kt_pool = ctx.enter_context(tc.tile_pool(name="kt", bufs=2))
        suf_pool = ctx.enter_context(tc.tile_pool(name="suf", bufs=2))

        with tc.tile_pool(name="pps", bufs=2, space="PSUM") as pps_pool, \
             tc.tile_pool(name="sps", bufs=2, space="PSUM") as sps_pool, \
             tc.tile_pool(name="ops", bufs=2, space="PSUM") as ops_pool, \
             tc.tile_pool(name="esb", bufs=4) as e_pool, \
             tc.tile_pool(name="zsb", bufs=2) as z_pool:

            def emit_kproj_kg(pr, KT_sb, kg):
                ps = pps_pool.tile([128, 512], F32, tag="pps")
                for c in range(NCH):
                    nc.tensor.matmul(
                        out=ps, lhsT=wk_sb[:, c, 128 * pr:128 * (pr + 1)],
                        rhs=xT_sb[:, c, 512 * kg:512 * (kg + 1)],
                        start=(c == 0), stop=(c == NCH - 1),
                    )
                nc.vector.tensor_scalar_add(
                    out=KT_sb[:, 512 * kg:512 * (kg + 1)], in0=ps,
                    scalar1=bk_sb[:, pr:pr + 1],
                )

            def emit_suf(pr):
                # psb[d, b] = w * sum_{tok in block b} V[tok, d]; suffix over b
                psb = pps_pool.tile([128, 4], F32, tag="pps")
                for c in range(NCH):
                    nc.tensor.matmul(
                        out=psb, lhsT=wv_sb[:, c, 128 * pr:128 * (pr + 1)],
                        rhs=xsum_sb[:, c, :],
                        start=(c == 0), stop=(c == NCH - 1),
                    )
                sufT = suf_pool.tile([64, 2, 4], F32)
                for hb in range(2):
                    hs = slice(64 * hb, 64 * hb + 64)
                    nc.vector.memset(sufT[:, hb, 3:4], 0.0)
                    nc.vector.tensor_copy(out=sufT[:, hb, 2:3], in_=psb[hs, 3:4])
                    nc.vector.tensor_add(
                        out=sufT[:, hb, 1:2], in0=psb[hs, 2:3], in1=sufT[:, hb, 2:3])
                    nc.vector.tensor_add(
                        out=sufT[:, hb, 0:1], in0=psb[hs, 1:2], in1=sufT[:, hb, 1:2])
                return sufT

            KT_cur = kt_pool.tile([128, S], BF16)
            for kg in range(4):
                emit_kproj_kg(0, KT_cur, kg)
            suf_cur = emit_suf(0)

            for pr in range(PAIRS):
                if pr + 1 < PAIRS:
                    KT_nxt = kt_pool.tile([128, S], BF16)
                else:
                    KT_nxt = None
                suf_nxt = None
                for j in range(NJ):
                    po = [None, None]
                    e_kb = [None, None]
                    for kb in range(j + 1):
                        diag = kb == j
                        # scores for both heads of the pair, row-tiled to run
                        # concurrently on the two 64-row halves of the PE
                        pss = [None, None]
                        for hl in range(2):
                            hsl = slice(64 * hl, 64 * (hl + 1))
                            pss[hl] = sps_pool.tile([128, 4, 256], F32, tag="sps", name="pss")
                            for s2 in range(4):
                                nc.tensor.matmul(
                                    out=pss[hl][:, s2, :],
                                    lhsT=KT_cur[hsl, 512 * kb + 128 * s2:512 * kb + 128 * (s2 + 1)],
                                    rhs=QT_all[hsl, pr, 256 * j:256 * (j + 1)],
                                    start=True, stop=True,
                                )
                        for hl in range(2):
                            if po[hl] is None:
                                po[hl] = ops_pool.tile([65, 512], F32, tag="ops", name="po")
                            e_sb = e_pool.tile([128, 4, 256], BF16, tag="e")
                            nc.scalar.activation(
                                out=e_sb, in_=pss[hl], func=AF.Exp, scale=0.125)
                            if diag:
                                nc.vector.copy_predicated(e_sb, maskp_sb, maskw_sb)
                            for s2 in range(4):
                                nc.tensor.matmul(
                                    out=po[hl][:, 0:256],
                                    lhsT=V_sb[:, 4 * kb + s2, 2 * pr + hl, :],
                                    rhs=e_sb[:, s2, :],
                                    start=(kb == 0 and s2 == 0),
                                    stop=(kb == j and s2 == 3),
                                    skip_group_check=True,
                                )
                        # PE filler: next pair's K projection
                        if kb == 0 and KT_nxt is not None:
                            emit_kproj_kg(pr + 1, KT_nxt, j)
                        if j == NJ - 1 and kb == 2 and KT_nxt is not None:
                            suf_nxt = emit_suf(pr + 1)
                    for hl in range(2):
                        zf = z_pool.tile([1, 256], F32, tag="zf")
                        nc.vector.tensor_scalar_add(
                            out=zf, in0=po[hl][64:65, 0:256],
                            scalar1=W_MASK * (S - KB * (j + 1)),
                        )
                        zi = z_pool.tile([1, 256], F32, tag="zi")
                        nc.vector.reciprocal_approx_fast(out=zi, in_=zf)
                        zr = z_pool.tile([1, 256], F32R, tag="zr")
                        nc.gpsimd.tensor_copy(out=zr, in_=zi)
                        nc.tensor.matmul(
                            out=po[hl][0:64, 256:512],
                            lhsT=ones_sb[0:1, 0:64], rhs=zr,
                            start=True, stop=True,
                        )
                        nm = z_pool.tile([64, 256], F32, tag="nm")
                        nc.vector.tensor_scalar_add(
                            out=nm, in0=po[hl][0:64, 0:256],
                            scalar1=suf_cur[:, hl, j:j + 1],
                        )
                        nc.vector.tensor_mul(
                            out=O_sb[64 * hl:64 * (hl + 1), pr, 256 * j:256 * (j + 1)],
                            in0=nm, in1=po[hl][0:64, 256:512],
                        )
                KT_cur = KT_nxt
                suf_cur = suf_nxt

        # ---------- output projection ----------
        with tc.tile_pool(name="fps", bufs=2, space="PSUM") as fps_pool, \
             tc.tile_pool(name="fout", bufs=3) as fo_pool:
            for jj in range(NJ):
                for dt_ in range(8):
                    ps = fps_pool.tile([128, 256], F32)
                    for c in range(NCH):
                        nc.tensor.matmul(
                            out=ps, lhsT=wo_sb[:, c, dt_, :],
                            rhs=O_sb[:, c, 256 * jj:256 * (jj + 1)],
                            start=(c == 0), stop=(c == NCH - 1),
                        )
                    fo = fo_pool.tile([128, 256], F32)
                    nc.vector.tensor_scalar_add(out=fo, in0=ps, scalar1=bo_sb[:, dt_:dt_ + 1])
                    nc.sync.dma_start(
                        out=outT[128 * dt_:128 * (dt_ + 1), 256 * jj:256 * (jj + 1)],
                        in_=fo,
                    )
    nc.compile()
    return nc


def qrows_for(p):
    return np.concatenate(
        [np.arange(QT * (2 * j + p), QT * (2 * j + p) + QT) for j in range(NJ)]
    )


def host_in_maps(x, Wqkv, bqkv, Wo, bo):
    import ml_dtypes
    bf16 = ml_dtypes.bfloat16

    x = np.asarray(x, np.float32)
    Wqkv = np.asarray(Wqkv, np.float32)
    bqkv = np.asarray(bqkv, np.float32)
    Wo = np.asarray(Wo, np.float32)
    bo = np.asarray(bo, np.float32)

    wq_h = np.ascontiguousarray(Wqkv[:, 0:D]).astype(bf16)
    wk_h = np.ascontiguousarray(Wqkv[:, D:2 * D]).astype(bf16)
    wv_h = np.ascontiguousarray(Wqkv[:, 2 * D:3 * D]).astype(bf16)
    wo_h = np.ascontiguousarray(Wo).astype(bf16)

    bq2 = np.ascontiguousarray(bqkv[0:D].reshape(8, 128).T)
    bk2 = np.ascontiguousarray(bqkv[D:2 * D].reshape(8, 128).T)
    # V bias folded into the output-projection bias: out = num/Z + bv -> @Wo
    bo_eff = bo + bqkv[2 * D:3 * D] @ Wo
    bo2 = np.ascontiguousarray(bo_eff.reshape(8, 128).T.astype(np.float32))
    onesd = np.ones((1, 512), np.float32)

    kap = np.arange(128)[:, None]
    r = np.arange(QT)[None, :]
    masks = {}
    for p in range(2):
        mm = np.zeros((128, 4, QT), np.float32)
        for s in range(4):
            mm[:, s, :] = (128 * s + kap <= QT * p + r)
        pred = np.ascontiguousarray(
            (1.0 - mm.reshape(128, 4 * QT)).astype(np.uint8))
        masks[p] = pred
    wdata = np.ascontiguousarray(
        np.full((128, 4 * QT), W_MASK, np.float32).astype(bf16))

    in_maps = []
    for core in range(8):
        b, p = core // 2, core % 2
        in_maps.append({
            "xT": np.ascontiguousarray(x[b].T).astype(bf16),
            "xqT": np.ascontiguousarray(x[b][qrows_for(p)].T).astype(bf16),
            "wq": wq_h,
            "wk": wk_h,
            "wv": wv_h,
            "wo": wo_h,
            "bq2": bq2,
            "bk2": bk2,
            "bo2": bo2,
            "maskw": wdata,
            "maskp": masks[p],
            "onesd": onesd,
        })
    return in_maps


_CACHED = {}


def get_program():
    if "nc" not in _CACHED:
        _CACHED["nc"] = build_program()
    return _CACHED["nc"]


def kernel(x, Wqkv, bqkv, Wo, bo):
    from concourse.bass_utils import run_bass_kernel_spmd

    nc = get_program()
    in_maps = host_in_maps(x, Wqkv, bqkv, Wo, bo)
    res = run_bass_kernel_spmd(nc, in_maps, core_ids=list(range(8)))
    out = np.zeros((B, S, D), np.float32)
    for core in range(8):
        b, p = core // 2, core % 2
        out[b, qrows_for(p), :] = res.results[core]["outT"].T
    return out


# revision 26
# speedup vs baseline: 1.4653x; 1.0715x over previous
"""Trainium2 Bass kernel for causal multi-head attention block (v2).

Reference computation (B=4, S=2048, D=1024, H=16, HD=64, fp32):
    qkv = x @ Wqkv + bqkv; split q,k,v; per-head scaled scores;
    causal mask filled with -0.0001 (leaky, NOT -inf); softmax over all
    2048 keys; out = P @ V; out = out @ Wo + bo.

Sharding: 8 cores, core = (batch b = i//2, parity p = i%2). Each core
computes 1024 queries of its batch: query tiles t = 2j+p (j=0..3) of
256 queries -> identical SPMD program, zero cross-core communication.

v2 design vs v1 baseline:
  - all inputs bf16 host-side (half DMA, FWL weight loads, 2x DVE).
  - score matmuls packed 2 heads/slot via PE row tiling (tile_position
    (0,0)/(64,0) auto-derived from 64-partition slices -> concurrent).
  - leaky diag mask via one copy_predicated (w at masked positions).
  - V bias folded into bo' = bo + bv @ Wo on host (exact).
  - per-qtile suffix correction (skipped blocks) and w*nskip Z term
    injected via a single K=1 rank-1 matmul into the PV PSUM group.
  - 1/8 score scale folded into the exp activation's scale field.
  - all weights resident in SBUF up front; K-proj of pair pr+1 emitted
    inside pair pr's attention loop as PE filler (keeps HAM warm).
  - attention output kept in SBUF (no DRAM roundtrip).
"""

import math
from contextlib import ExitStack

import numpy as np

import concourse.bass as bass
import concourse.mybir as mybir
import concourse.tile as tile
from concourse import bacc

F32 = mybir.dt.float32
F32R = mybir.dt.float32r
BF16 = mybir.dt.bfloat16
AF = mybir.ActivationFunctionType
ALU = mybir.AluOpType
AX = mybir.AxisListType

B, S, D, H, HD = 4, 2048, 1024, 16, 64
QL, QT, KB, NJ = 1024, 256, 512, 4    # queries/core, qtile, key block, n qtiles
NCH = D // 128                         # contraction chunks
PAIRS = H // 2
W_MASK = math.exp(-1e-4)


def build_program():
    nc = bacc.Bacc(
        "TRN2",
        target_bir_lowering=False,
        debug=False,
        num_devices=8,
    )
    xT = nc.declare_dram_parameter("xT", [D, S], BF16, isOutput=False)
    xqT = nc.declare_dram_parameter("xqT", [D, QL], BF16, isOutput=False)
    wq = nc.declare_dram_parameter("wq", [D, D], BF16, isOutput=False)
    wk = nc.declare_dram_parameter("wk", [D, D], BF16, isOutput=False)
    wv = nc.declare_dram_parameter("wv", [D, D], BF16, isOutput=False)
    wo = nc.declare_dram_parameter("wo", [D, D], BF16, isOutput=False)
    bq2 = nc.declare_dram_parameter("bq2", [128, 8], F32, isOutput=False)
    bk2 = nc.declare_dram_parameter("bk2", [128, 8], F32, isOutput=False)
    bo2 = nc.declare_dram_parameter("bo2", [128, 8], F32, isOutput=False)
    maskw = nc.declare_dram_parameter("maskw", [128, 4 * QT], BF16, isOutput=False)
    maskp = nc.declare_dram_parameter("maskp", [128, 4 * QT], mybir.dt.uint8, isOutput=False)
    onesd = nc.declare_dram_parameter("onesd", [1, 512], F32R, isOutput=False)
    outT = nc.declare_dram_parameter("outT", [D, QL], F32, isOutput=True)

    with tile.TileContext(nc) as tc, ExitStack() as ctx, \
         nc.allow_low_precision(reason="bf16 matmul inputs within rel-err budget"):
        consts = ctx.enter_context(tc.tile_pool(name="consts", bufs=1))
        bq_sb = consts.tile([128, 8], F32)
        nc.sync.dma_start(out=bq_sb, in_=bq2[:])
        bk_sb = consts.tile([128, 8], F32)
        nc.sync.dma_start(out=bk_sb, in_=bk2[:])
        bo_sb = consts.tile([128, 8], F32)
        nc.sync.dma_start(out=bo_sb, in_=bo2[:])
        maskw_sb = consts.tile([128, 4, QT], BF16)
        nc.sync.dma_start(out=maskw_sb, in_=maskw[:].rearrange("p (a b) -> p a b", a=4))
        maskp_sb = consts.tile([128, 4, QT], mybir.dt.uint8)
        nc.sync.dma_start(out=maskp_sb, in_=maskp[:].rearrange("p (a b) -> p a b", a=4))
        ones_sb = consts.tile([1, 512], F32R)
        nc.sync.dma_start(out=ones_sb, in_=onesd[:])

        # ---------- resident activations + weights ----------
        # DMA order: Q-projection inputs first so the PE can start ASAP.
        w_pool = ctx.enter_context(tc.tile_pool(name="wsb", bufs=1))
        wq_sb = w_pool.tile([128, NCH, D], BF16)
        nc.sync.dma_start(out=wq_sb, in_=wq[:].rearrange("(c p) m -> p c m", p=128))
        xt_pool = ctx.enter_context(tc.tile_pool(name="xt", bufs=1))
        xT_sb = xt_pool.tile([128, NCH, S], BF16)
        wv_sb = w_pool.tile([128, NCH, D], BF16)
        wk_sb = w_pool.tile([128, NCH, D], BF16)
        wo_sb = w_pool.tile([128, NCH, 8, 128], BF16)

        xsum_sb = consts.tile([128, NCH, 4], BF16)

        qt_pool = ctx.enter_context(tc.tile_pool(name="qt", bufs=1))
        QT_all = qt_pool.tile([128, PAIRS, QL], BF16)
        v_pool = ctx.enter_context(tc.tile_pool(name="vsb", bufs=1))
        V_sb = v_pool.tile([128, 16, H, 65], BF16)
        nc.gpsimd.memset(V_sb[:, :, :, 64:65], 1.0)
        o_pool = ctx.enter_context(tc.tile_pool(name="osb", bufs=1))
        O_sb = o_pool.tile([128, NCH, QL], BF16)

        # ---------- Q projection (queries for this core only) ----------
        with tc.tile_pool(name="xq", bufs=1) as xq_pool, \
             tc.tile_pool(name="qps", bufs=2, space="PSUM") as qps_pool:
            xqT_sb = xq_pool.tile([128, NCH, QL], BF16)
            for c in range(NCH):
                nc.sync.dma_start(out=xqT_sb[:, c, :], in_=xqT[128 * c:128 * (c + 1), :])
            for c in range(NCH):
                nc.sync.dma_start(out=xT_sb[:, c, :], in_=xT[128 * c:128 * (c + 1), :])
            nc.sync.dma_start(out=wv_sb, in_=wv[:].rearrange("(c p) m -> p c m", p=128))
            nc.sync.dma_start(out=wk_sb, in_=wk[:].rearrange("(c p) m -> p c m", p=128))
            nc.sync.dma_start(
                out=wo_sb, in_=wo[:].rearrange("(c p) (t m) -> p c t m", p=128, m=128)
            )
            # per-512-block column sums of xT, pre-scaled by w (V suffix sums)
            for c in range(NCH):
                nc.vector.tensor_reduce(
                    out=xsum_sb[:, c, :],
                    in_=xT_sb[:, c, :].rearrange("p (b t) -> p b t", b=4),
                    axis=AX.X, op=ALU.add,
                )
            nc.gpsimd.tensor_scalar_mul(
                out=xsum_sb[:].rearrange("p c b -> p (c b)"),
                in0=xsum_sb[:].rearrange("p c b -> p (c b)"),
                scalar1=W_MASK,
            )
            for pr in range(PAIRS):
                for g2 in range(2):
                    ps = qps_pool.tile([128, 512], F32)
                    for c in range(NCH):
                        nc.tensor.matmul(
                            out=ps, lhsT=wq_sb[:, c, 128 * pr:128 * (pr + 1)],
                            rhs=xqT_sb[:, c, 512 * g2:512 * (g2 + 1)],
                            start=(c == 0), stop=(c == NCH - 1),
                        )
                    nc.scalar.activation(
                        out=QT_all[:, pr, 512 * g2:512 * (g2 + 1)], in_=ps,
                        func=AF.Identity, bias=bq_sb[:, pr:pr + 1],
                    )

        # ---------- V projection, all heads (token-major, 65th ones col) ----
        with tc.tile_pool(name="vps", bufs=2, space="PSUM") as vps_pool:
            for g in range(4):
                for t in range(16):
                    ps = vps_pool.tile([128, 256], F32)
                    for c in range(NCH):
                        nc.tensor.matmul(
                            out=ps, lhsT=xT_sb[:, c, 128 * t:128 * (t + 1)],
                            rhs=wv_sb[:, c, 256 * g:256 * (g + 1)],
                            start=(c == 0), stop=(c == NCH - 1),
                        )
                    nc.scalar.activation(
                        out=V_sb[:, t, 4 * g:4 * (g + 1), 0:64],
                        in_=ps.rearrange("p (h d) -> p h d", h=4),
                        func=AF.Identity,
                    )

        # ---------- main loop over head pairs ----------
        kt_pool = ctx.enter_context(tc.tile_pool(name="kt", bufs=2))
        suf_pool = ctx.enter_context(tc.tile_pool(name="suf", bufs=2))

        with tc.tile_pool(name="pps", bufs=2, space="PSUM") as pps_pool, \
             tc.tile_pool(name="sps", bufs=2, space="PSUM") as sps_pool, \
             tc.tile_pool(name="ops", bufs=2, space="PSUM") as ops_pool, \
             tc.tile_pool(name="esb", bufs=4) as e_pool, \
             tc.tile_pool(name="zsb", bufs=2) as z_pool:

            def emit_kproj_kg(pr, KT_sb, kg):
                ps = pps_pool.tile([128, 512], F32, tag="pps")
                for c in range(NCH):
                    nc.tensor.matmul(
                        out=ps, lhsT=wk_sb[:, c, 128 * pr:128 * (pr + 1)],
                        rhs=xT_sb[:, c, 512 * kg:512 * (kg + 1)],
                        start=(c == 0), stop=(c == NCH - 1),
                    )
                nc.scalar.activation(
                    out=KT_sb[:, 512 * kg:512 * (kg + 1)], in_=ps,
                    func=AF.Identity, bias=bk_sb[:, pr:pr + 1],
                )

            def emit_suf(pr):
                # psb[d, b] = w * sum_{tok in block b} V[tok, d]; suffix over b
                psb = pps_pool.tile([128, 4], F32, tag="pps")
                for c in range(NCH):
                    nc.tensor.matmul(
                        out=psb, lhsT=wv_sb[:, c, 128 * pr:128 * (pr + 1)],
                        rhs=xsum_sb[:, c, :],
                        start=(c == 0), stop=(c == NCH - 1),
                    )
                sufT = suf_pool.tile([64, 2, 4], F32)
                for hb in range(2):
                    hs = slice(64 * hb, 64 * hb + 64)
                    nc.vector.memset(sufT[:, hb, 3:4], 0.0)
                    nc.vector.tensor_copy(out=sufT[:, hb, 2:3], in_=psb[hs, 3:4])
                    nc.vector.tensor_add(
                        out=sufT[:, hb, 1:2], in0=psb[hs, 2:3], in1=sufT[:, hb, 2:3])
                    nc.vector.tensor_add(
                        out=sufT[:, hb, 0:1], in0=psb[hs, 1:2], in1=sufT[:, hb, 1:2])
                return sufT

            KT_cur = kt_pool.tile([128, S], BF16)
            for kg in range(4):
                emit_kproj_kg(0, KT_cur, kg)
            suf_cur = emit_suf(0)

            for pr in range(PAIRS):
                if pr + 1 < PAIRS:
                    KT_nxt = kt_pool.tile([128, S], BF16)
                else:
                    KT_nxt = None
                suf_nxt = None
                for j in range(NJ):
                    po = [None, None]
                    e_kb = [None, None]
                    for kb in range(j + 1):
                        diag = kb == j
                        # scores for both heads of the pair, row-tiled to run
                        # concurrently on the two 64-row halves of the PE
                        pss = [None, None]
                        for hl in range(2):
                            hsl = slice(64 * hl, 64 * (hl + 1))
                            pss[hl] = sps_pool.tile([128, 4, 256], F32, tag="sps", name="pss")
                            for s2 in range(4):
                                nc.tensor.matmul(
                                    out=pss[hl][:, s2, :],
                                    lhsT=KT_cur[hsl, 512 * kb + 128 * s2:512 * kb + 128 * (s2 + 1)],
                                    rhs=QT_all[hsl, pr, 256 * j:256 * (j + 1)],
                                    start=True, stop=True,
                                )
                        for hl in range(2):
                            if po[hl] is None:
                                po[hl] = ops_pool.tile([65, 512], F32, tag="ops", name="po")
                            e_sb = e_pool.tile([128, 4, 256], BF16, tag="e")
                            nc.scalar.activation(
                                out=e_sb, in_=pss[hl], func=AF.Exp, scale=0.125)
                            if diag:
                                nc.vector.copy_predicated(e_sb, maskp_sb, maskw_sb)
                            for s2 in range(4):
                                nc.tensor.matmul(
                                    out=po[hl][:, 0:256],
                                    lhsT=V_sb[:, 4 * kb + s2, 2 * pr + hl, :],
                                    rhs=e_sb[:, s2, :],
                                    start=(kb == 0 and s2 == 0),
                                    stop=(kb == j and s2 == 3),
                                    skip_group_check=True,
                                )
                        # PE filler: next pair's K projection
                        if kb == 0 and KT_nxt is not None:
                            emit_kproj_kg(pr + 1, KT_nxt, j)
                        if j == NJ - 1 and kb == 2 and KT_nxt is not None:
                            suf_nxt = emit_suf(pr + 1)
                    for hl in range(2):
                        zf = z_pool.tile([1, 256], F32, tag="zf")
                        nc.vector.tensor_scalar_add(
                            out=zf, in0=po[hl][64:65, 0:256],
                            scalar1=W_MASK * (S - KB * (j + 1)),
                        )
                        zi = z_pool.tile([1, 256], F32, tag="zi")
                        nc.vector.reciprocal_approx_fast(out=zi, in_=zf)
                        zr = z_pool.tile([1, 256], F32R, tag="zr")
                        nc.gpsimd.tensor_copy(out=zr, in_=zi)
                        nc.tensor.matmul(
                            out=po[hl][0:64, 256:512],
                            lhsT=ones_sb[0:1, 0:64], rhs=zr,
                            start=True, stop=True,
                        )
                        nm = z_pool.tile([64, 256], F32, tag="nm")
                        nc.vector.tensor_scalar_add(
                            out=nm, in0=po[hl][0:64, 0:256],
                            scalar1=suf_cur[:, hl, j:j + 1],
                        )
                        nc.vector.tensor_mul(
                            out=O_sb[64 * hl:64 * (hl + 1), pr, 256 * j:256 * (j + 1)],
                            in0=nm, in1=po[hl][0:64, 256:512],
                        )
                KT_cur = KT_nxt
                suf_cur = suf_nxt

        # ---------- output projection ----------
        with tc.tile_pool(name="fps", bufs=2, space="PSUM") as fps_pool, \
             tc.tile_pool(name="fout", bufs=3) as fo_pool:
            for jj in range(NJ):
                for dt_ in range(8):
                    ps = fps_pool.tile([128, 256], F32)
                    for c in range(NCH):
                        nc.tensor.matmul(
                            out=ps, lhsT=wo_sb[:, c, dt_, :],
                            rhs=O_sb[:, c, 256 * jj:256 * (jj + 1)],
                            start=(c == 0), stop=(c == NCH - 1),
                        )
                    fo = fo_pool.tile([128, 256], F32)
                    nc.scalar.activation(
                        out=fo, in_=ps, func=AF.Identity, bias=bo_sb[:, dt_:dt_ + 1])
                    nc.sync.dma_start(
                        out=outT[128 * dt_:128 * (dt_ + 1), 256 * jj:256 * (jj + 1)],
                        in_=fo,
                    )
    nc.compile()
    return nc


def qrows_for(p):
    return np.concatenate(
        [np.arange(QT * (2 * j + p), QT * (2 * j + p) + QT) for j in range(NJ)]
    )


def host_in_maps(x, Wqkv, bqkv, Wo, bo):
    import ml_dtypes
    bf16 = ml_dtypes.bfloat16

    x = np.asarray(x, np.float32)
    Wqkv = np.asarray(Wqkv, np.float32)
    bqkv = np.asarray(bqkv, np.float32)
    Wo = np.asarray(Wo, np.float32)
    bo = np.asarray(bo, np.float32)

    wq_h = np.ascontiguousarray(Wqkv[:, 0:D]).astype(bf16)
    wk_h = np.ascontiguousarray(Wqkv[:, D:2 * D]).astype(bf16)
    wv_h = np.ascontiguousarray(Wqkv[:, 2 * D:3 * D]).astype(bf16)
    wo_h = np.ascontiguousarray(Wo).astype(bf16)

    bq2 = np.ascontiguousarray(bqkv[0:D].reshape(8, 128).T)
    bk2 = np.ascontiguousarray(bqkv[D:2 * D].reshape(8, 128).T)
    # V bias folded into the output-projection bias: out = num/Z + bv -> @Wo
    bo_eff = bo + bqkv[2 * D:3 * D] @ Wo
    bo2 = np.ascontiguousarray(bo_eff.reshape(8, 128).T.astype(np.float32))
    onesd = np.ones((1, 512), np.float32)

    kap = np.arange(128)[:, None]
    r = np.arange(QT)[None, :]
    masks = {}
    for p in range(2):
        mm = np.zeros((128, 4, QT), np.float32)
        for s in range(4):
            mm[:, s, :] = (128 * s + kap <= QT * p + r)
        pred = np.ascontiguousarray(
            (1.0 - mm.reshape(128, 4 * QT)).astype(np.uint8))
        masks[p] = pred
    wdata = np.ascontiguousarray(
        np.full((128, 4 * QT), W_MASK, np.float32).astype(bf16))

    in_maps = []
    for core in range(8):
        b, p = core // 2, core % 2
        in_maps.append({
            "xT": np.ascontiguousarray(x[b].T).astype(bf16),
            "xqT": np.ascontiguousarray(x[b][qrows_for(p)].T).astype(bf16),
            "wq": wq_h,
            "wk": wk_h,
            "wv": wv_h,
            "wo": wo_h,
            "bq2": bq2,
            "bk2": bk2,
            "bo2": bo2,
            "maskw": wdata,
            "maskp": masks[p],
            "onesd": onesd,
        })
    return in_maps


_CACHED = {}


def get_program():
    if "nc" not in _CACHED:
        _CACHED["nc"] = build_program()
    return _CACHED["nc"]


def kernel(x, Wqkv, bqkv, Wo, bo):
    from concourse.bass_utils import run_bass_kernel_spmd

    nc = get_program()
    in_maps = host_in_maps(x, Wqkv, bqkv, Wo, bo)
    res = run_bass_kernel_spmd(nc, in_maps, core_ids=list(range(8)))
    out = np.zeros((B, S, D), np.float32)
    for core in range(8):
        b, p = core // 2, core % 2
        out[b, qrows_for(p), :] = res.results[core]["outT"].T
    return out


# revision 29
# speedup vs baseline: 1.6631x; 1.1350x over previous
"""Trainium2 Bass kernel for causal multi-head attention block (v2).

Reference computation (B=4, S=2048, D=1024, H=16, HD=64, fp32):
    qkv = x @ Wqkv + bqkv; split q,k,v; per-head scaled scores;
    causal mask filled with -0.0001 (leaky, NOT -inf); softmax over all
    2048 keys; out = P @ V; out = out @ Wo + bo.

Sharding: 8 cores, core = (batch b = i//2, parity p = i%2). Each core
computes 1024 queries of its batch: query tiles t = 2j+p (j=0..3) of
256 queries -> identical SPMD program, zero cross-core communication.

v2 design vs v1 baseline:
  - all inputs bf16 host-side (half DMA, FWL weight loads, 2x DVE).
  - score matmuls packed 2 heads/slot via PE row tiling (tile_position
    (0,0)/(64,0) auto-derived from 64-partition slices -> concurrent).
  - leaky diag mask via one copy_predicated (w at masked positions).
  - V bias folded into bo' = bo + bv @ Wo on host (exact).
  - per-qtile suffix correction (skipped blocks) and w*nskip Z term
    injected via a single K=1 rank-1 matmul into the PV PSUM group.
  - 1/8 score scale folded into the exp activation's scale field.
  - all weights resident in SBUF up front; K-proj of pair pr+1 emitted
    inside pair pr's attention loop as PE filler (keeps HAM warm).
  - attention output kept in SBUF (no DRAM roundtrip).
"""

import math
from contextlib import ExitStack

import numpy as np

import concourse.bass as bass
import concourse.mybir as mybir
import concourse.tile as tile
from concourse import bacc

F32 = mybir.dt.float32
F32R = mybir.dt.float32r
BF16 = mybir.dt.bfloat16
AF = mybir.ActivationFunctionType
ALU = mybir.AluOpType
AX = mybir.AxisListType

B, S, D, H, HD = 4, 2048, 1024, 16, 64
QL, QT, KB, NJ = 1024, 256, 512, 4    # queries/core, qtile, key block, n qtiles
NCH = D // 128                         # contraction chunks
PAIRS = H // 2
W_MASK = math.exp(-1e-4)


def build_program():
    nc = bacc.Bacc(
        "TRN2",
        target_bir_lowering=False,
        debug=False,
        num_devices=8,
    )
    xT = nc.declare_dram_parameter("xT", [D, S], BF16, isOutput=False)
    xqT = nc.declare_dram_parameter("xqT", [D, QL], BF16, isOutput=False)
    wq = nc.declare_dram_parameter("wq", [D, D], BF16, isOutput=False)
    wk = nc.declare_dram_parameter("wk", [D, D], BF16, isOutput=False)
    wv = nc.declare_dram_parameter("wv", [D, D], BF16, isOutput=False)
    wo = nc.declare_dram_parameter("wo", [D, D], BF16, isOutput=False)
    bq2 = nc.declare_dram_parameter("bq2", [128, 8], F32, isOutput=False)
    bk2 = nc.declare_dram_parameter("bk2", [128, 8], F32, isOutput=False)
    bo2 = nc.declare_dram_parameter("bo2", [128, 8], F32, isOutput=False)
    maskw = nc.declare_dram_parameter("maskw", [128, 4 * QT], BF16, isOutput=False)
    maskp = nc.declare_dram_parameter("maskp", [128, 4 * QT], mybir.dt.uint8, isOutput=False)
    onesd = nc.declare_dram_parameter("onesd", [1, 512], F32R, isOutput=False)
    outT = nc.declare_dram_parameter("outT", [D, QL], F32, isOutput=True)

    with tile.TileContext(nc) as tc, ExitStack() as ctx, \
         nc.allow_low_precision(reason="bf16 matmul inputs within rel-err budget"):
        consts = ctx.enter_context(tc.tile_pool(name="consts", bufs=1))
        bq_sb = consts.tile([128, 8], F32)
        nc.sync.dma_start(out=bq_sb, in_=bq2[:])
        bk_sb = consts.tile([128, 8], F32)
        nc.sync.dma_start(out=bk_sb, in_=bk2[:])
        bo_sb = consts.tile([128, 8], F32)
        nc.sync.dma_start(out=bo_sb, in_=bo2[:])
        maskw_sb = consts.tile([128, 4, QT], BF16)
        nc.sync.dma_start(out=maskw_sb, in_=maskw[:].rearrange("p (a b) -> p a b", a=4))
        maskp_sb = consts.tile([128, 4, QT], mybir.dt.uint8)
        nc.sync.dma_start(out=maskp_sb, in_=maskp[:].rearrange("p (a b) -> p a b", a=4))
        ones_sb = consts.tile([1, 512], F32R)
        nc.sync.dma_start(out=ones_sb, in_=onesd[:])

        # ---------- resident activations + weights ----------
        # DMA order: Q-projection inputs first so the PE can start ASAP.
        w_pool = ctx.enter_context(tc.tile_pool(name="wsb", bufs=1))
        wq_sb = w_pool.tile([128, NCH, D], BF16)
        nc.sync.dma_start(out=wq_sb, in_=wq[:].rearrange("(c p) m -> p c m", p=128))
        xt_pool = ctx.enter_context(tc.tile_pool(name="xt", bufs=1))
        xT_sb = xt_pool.tile([128, NCH, S], BF16)
        wv_sb = w_pool.tile([128, NCH, D], BF16)
        wk_sb = w_pool.tile([128, NCH, D], BF16)
        wo_sb = w_pool.tile([128, NCH, 8, 128], BF16)

        xsum_sb = consts.tile([128, NCH, 4], BF16)

        qt_pool = ctx.enter_context(tc.tile_pool(name="qt", bufs=1))
        QT_all = qt_pool.tile([128, PAIRS, QL], BF16)
        v_pool = ctx.enter_context(tc.tile_pool(name="vsb", bufs=1))
        V_sb = v_pool.tile([128, 16, H, 65], BF16)
        nc.gpsimd.memset(V_sb[:, :, :, 64:65], 1.0)
        o_pool = ctx.enter_context(tc.tile_pool(name="osb", bufs=1))
        O_sb = o_pool.tile([128, NCH, QL], BF16)

        # ---------- Q projection (queries for this core only) ----------
        with tc.tile_pool(name="xq", bufs=1) as xq_pool, \
             tc.tile_pool(name="qps", bufs=2, space="PSUM") as qps_pool:
            xqT_sb = xq_pool.tile([128, NCH, QL], BF16)
            for c in range(NCH):
                nc.sync.dma_start(out=xqT_sb[:, c, :], in_=xqT[128 * c:128 * (c + 1), :])
            for c in range(NCH):
                nc.sync.dma_start(out=xT_sb[:, c, :], in_=xT[128 * c:128 * (c + 1), :])
            nc.sync.dma_start(out=wv_sb, in_=wv[:].rearrange("(c p) m -> p c m", p=128))
            nc.sync.dma_start(out=wk_sb, in_=wk[:].rearrange("(c p) m -> p c m", p=128))
            nc.sync.dma_start(
                out=wo_sb, in_=wo[:].rearrange("(c p) (t m) -> p c t m", p=128, m=128)
            )
            # per-512-block column sums of xT, pre-scaled by w (V suffix sums)
            for c in range(NCH):
                nc.vector.tensor_reduce(
                    out=xsum_sb[:, c, :],
                    in_=xT_sb[:, c, :].rearrange("p (b t) -> p b t", b=4),
                    axis=AX.X, op=ALU.add,
                )
            nc.gpsimd.tensor_scalar_mul(
                out=xsum_sb[:].rearrange("p c b -> p (c b)"),
                in0=xsum_sb[:].rearrange("p c b -> p (c b)"),
                scalar1=W_MASK,
            )
            for pr in range(PAIRS):
                for g2 in range(2):
                    ps = qps_pool.tile([128, 512], F32)
                    for c in range(NCH):
                        nc.tensor.matmul(
                            out=ps, lhsT=wq_sb[:, c, 128 * pr:128 * (pr + 1)],
                            rhs=xqT_sb[:, c, 512 * g2:512 * (g2 + 1)],
                            start=(c == 0), stop=(c == NCH - 1),
                        )
                    nc.scalar.activation(
                        out=QT_all[:, pr, 512 * g2:512 * (g2 + 1)], in_=ps,
                        func=AF.Identity, bias=bq_sb[:, pr:pr + 1],
                    )

        # ---------- V projection, all heads (token-major, 65th ones col) ----
        with tc.tile_pool(name="vps", bufs=2, space="PSUM") as vps_pool:
            for g in range(2):
                for t in range(16):
                    ps = vps_pool.tile([128, 512], F32)
                    for c in range(NCH):
                        nc.tensor.matmul(
                            out=ps, lhsT=xT_sb[:, c, 128 * t:128 * (t + 1)],
                            rhs=wv_sb[:, c, 512 * g:512 * (g + 1)],
                            start=(c == 0), stop=(c == NCH - 1),
                        )
                    nc.scalar.activation(
                        out=V_sb[:, t, 8 * g:8 * (g + 1), 0:64],
                        in_=ps.rearrange("p (h d) -> p h d", h=8),
                        func=AF.Identity,
                    )

        # ---------- main loop over head pairs ----------
        kt_pool = ctx.enter_context(tc.tile_pool(name="kt", bufs=2))
        suf_pool = ctx.enter_context(tc.tile_pool(name="suf", bufs=2))

        with tc.tile_pool(name="pps", bufs=2, space="PSUM") as pps_pool, \
             tc.tile_pool(name="sps", bufs=2, space="PSUM") as sps_pool, \
             tc.tile_pool(name="ops", bufs=2, space="PSUM") as ops_pool, \
             tc.tile_pool(name="esb", bufs=4) as e_pool, \
             tc.tile_pool(name="zsb", bufs=2) as z_pool, \
             tc.tile_pool(name="fout", bufs=3) as fo_pool:

            def emit_kproj_kg(pr, KT_sb, kg):
                ps = pps_pool.tile([128, 512], F32, tag="pps")
                for c in range(NCH):
                    nc.tensor.matmul(
                        out=ps, lhsT=wk_sb[:, c, 128 * pr:128 * (pr + 1)],
                        rhs=xT_sb[:, c, 512 * kg:512 * (kg + 1)],
                        start=(c == 0), stop=(c == NCH - 1),
                    )
                nc.scalar.activation(
                    out=KT_sb[:, 512 * kg:512 * (kg + 1)], in_=ps,
                    func=AF.Identity, bias=bk_sb[:, pr:pr + 1],
                )

            def emit_suf(pr):
                # psb[d, b] = w * sum_{tok in block b} V[tok, d]; suffix over b
                psb = pps_pool.tile([128, 4], F32, tag="pps")
                for c in range(NCH):
                    nc.tensor.matmul(
                        out=psb, lhsT=wv_sb[:, c, 128 * pr:128 * (pr + 1)],
                        rhs=xsum_sb[:, c, :],
                        start=(c == 0), stop=(c == NCH - 1),
                    )
                sufT = suf_pool.tile([64, 2, 4], F32)
                for hb in range(2):
                    hs = slice(64 * hb, 64 * hb + 64)
                    nc.vector.memset(sufT[:, hb, 3:4], 0.0)
                    nc.vector.tensor_copy(out=sufT[:, hb, 2:3], in_=psb[hs, 3:4])
                    nc.vector.tensor_add(
                        out=sufT[:, hb, 1:2], in0=psb[hs, 2:3], in1=sufT[:, hb, 2:3])
                    nc.vector.tensor_add(
                        out=sufT[:, hb, 0:1], in0=psb[hs, 1:2], in1=sufT[:, hb, 1:2])
                return sufT

            def emit_outproj(jj):
                for dt_ in range(8):
                    ps = pps_pool.tile([128, 256], F32, tag="pps", name="fps")
                    for c in range(NCH):
                        nc.tensor.matmul(
                            out=ps, lhsT=wo_sb[:, c, dt_, :],
                            rhs=O_sb[:, c, 256 * jj:256 * (jj + 1)],
                            start=(c == 0), stop=(c == NCH - 1),
                        )
                    fo = fo_pool.tile([128, 256], F32, name="fo")
                    nc.scalar.activation(
                        out=fo, in_=ps, func=AF.Identity, bias=bo_sb[:, dt_:dt_ + 1])
                    nc.sync.dma_start(
                        out=outT[128 * dt_:128 * (dt_ + 1), 256 * jj:256 * (jj + 1)],
                        in_=fo,
                    )

            KT_cur = kt_pool.tile([128, S], BF16)
            for kg in range(4):
                emit_kproj_kg(0, KT_cur, kg)
            suf_cur = emit_suf(0)

            for pr in range(PAIRS):
                if pr + 1 < PAIRS:
                    KT_nxt = kt_pool.tile([128, S], BF16)
                else:
                    KT_nxt = None
                suf_nxt = None
                for j in range(NJ):
                    po = [None, None]
                    e_kb = [None, None]
                    for kb in range(j + 1):
                        diag = kb == j
                        # scores for both heads of the pair, row-tiled to run
                        # concurrently on the two 64-row halves of the PE
                        e_kb2 = [None, None]
                        for hl in range(2):
                            hsl = slice(64 * hl, 64 * (hl + 1))
                            pss = sps_pool.tile([128, 4, 256], F32, tag="sps", name="pss")
                            for s2 in range(4):
                                nc.tensor.matmul(
                                    out=pss[:, s2, :],
                                    lhsT=KT_cur[hsl, 512 * kb + 128 * s2:512 * kb + 128 * (s2 + 1)],
                                    rhs=QT_all[hsl, pr, 256 * j:256 * (j + 1)],
                                    start=True, stop=True,
                                )
                            e_sb = e_pool.tile([128, 4, 256], BF16, tag="e")
                            nc.scalar.activation(
                                out=e_sb, in_=pss, func=AF.Exp, scale=0.125)
                            if diag:
                                nc.vector.copy_predicated(e_sb, maskp_sb, maskw_sb)
                            e_kb2[hl] = e_sb
                        for hl in range(2):
                            if po[hl] is None:
                                po[hl] = ops_pool.tile([65, 512], F32, tag="ops", name="po")
                            for s2 in range(4):
                                nc.tensor.matmul(
                                    out=po[hl][:, 0:256],
                                    lhsT=V_sb[:, 4 * kb + s2, 2 * pr + hl, :],
                                    rhs=e_kb2[hl][:, s2, :],
                                    start=(kb == 0 and s2 == 0),
                                    stop=(kb == j and s2 == 3),
                                    skip_group_check=True,
                                )
                        # PE filler: next pair's K projection
                        if kb == 0 and KT_nxt is not None:
                            emit_kproj_kg(pr + 1, KT_nxt, j)
                        if j == NJ - 1 and kb == 2 and KT_nxt is not None:
                            suf_nxt = emit_suf(pr + 1)
                    for hl in range(2):
                        zf = z_pool.tile([1, 256], F32, tag="zf")
                        nc.vector.tensor_scalar_add(
                            out=zf, in0=po[hl][64:65, 0:256],
                            scalar1=W_MASK * (S - KB * (j + 1)),
                        )
                        zi = z_pool.tile([1, 256], F32, tag="zi")
                        nc.vector.reciprocal_approx_fast(out=zi, in_=zf)
                        zr = z_pool.tile([1, 256], F32R, tag="zr")
                        nc.vector.tensor_copy(out=zr, in_=zi)
                        nc.tensor.matmul(
                            out=po[hl][0:64, 256:512],
                            lhsT=ones_sb[0:1, 0:64], rhs=zr,
                            start=True, stop=True,
                        )
                        nm = z_pool.tile([64, 256], F32, tag="nm")
                        nc.vector.tensor_scalar_add(
                            out=nm, in0=po[hl][0:64, 0:256],
                            scalar1=suf_cur[:, hl, j:j + 1],
                        )
                        nc.vector.tensor_mul(
                            out=O_sb[64 * hl:64 * (hl + 1), pr, 256 * j:256 * (j + 1)],
                            in0=nm, in1=po[hl][0:64, 256:512],
                        )
                    if pr == PAIRS - 1:
                        emit_outproj(j)
                KT_cur = KT_nxt
                suf_cur = suf_nxt

    nc.compile()
    return nc


def qrows_for(p):
    return np.concatenate(
        [np.arange(QT * (2 * j + p), QT * (2 * j + p) + QT) for j in range(NJ)]
    )


def host_in_maps(x, Wqkv, bqkv, Wo, bo):
    import ml_dtypes
    bf16 = ml_dtypes.bfloat16

    x = np.asarray(x, np.float32)
    Wqkv = np.asarray(Wqkv, np.float32)
    bqkv = np.asarray(bqkv, np.float32)
    Wo = np.asarray(Wo, np.float32)
    bo = np.asarray(bo, np.float32)

    wq_h = np.ascontiguousarray(Wqkv[:, 0:D]).astype(bf16)
    wk_h = np.ascontiguousarray(Wqkv[:, D:2 * D]).astype(bf16)
    wv_h = np.ascontiguousarray(Wqkv[:, 2 * D:3 * D]).astype(bf16)
    wo_h = np.ascontiguousarray(Wo).astype(bf16)

    bq2 = np.ascontiguousarray(bqkv[0:D].reshape(8, 128).T)
    bk2 = np.ascontiguousarray(bqkv[D:2 * D].reshape(8, 128).T)
    # V bias folded into the output-projection bias: out = num/Z + bv -> @Wo
    bo_eff = bo + bqkv[2 * D:3 * D] @ Wo
    bo2 = np.ascontiguousarray(bo_eff.reshape(8, 128).T.astype(np.float32))
    onesd = np.ones((1, 512), np.float32)

    kap = np.arange(128)[:, None]
    r = np.arange(QT)[None, :]
    masks = {}
    for p in range(2):
        mm = np.zeros((128, 4, QT), np.float32)
        for s in range(4):
            mm[:, s, :] = (128 * s + kap <= QT * p + r)
        pred = np.ascontiguousarray(
            (1.0 - mm.reshape(128, 4 * QT)).astype(np.uint8))
        masks[p] = pred
    wdata = np.ascontiguousarray(
        np.full((128, 4 * QT), W_MASK, np.float32).astype(bf16))

    in_maps = []
    for core in range(8):
        b, p = core // 2, core % 2
        in_maps.append({
            "xT": np.ascontiguousarray(x[b].T).astype(bf16),
            "xqT": np.ascontiguousarray(x[b][qrows_for(p)].T).astype(bf16),
            "wq": wq_h,
            "wk": wk_h,
            "wv": wv_h,
            "wo": wo_h,
            "bq2": bq2,
            "bk2": bk2,
            "bo2": bo2,
            "maskw": wdata,
            "maskp": masks[p],
            "onesd": onesd,
        })
    return in_maps


_CACHED = {}


def get_program():
    if "nc" not in _CACHED:
        _CACHED["nc"] = build_program()
    return _CACHED["nc"]


def kernel(x, Wqkv, bqkv, Wo, bo):
    from concourse.bass_utils import run_bass_kernel_spmd

    nc = get_program()
    in_maps = host_in_maps(x, Wqkv, bqkv, Wo, bo)
    res = run_bass_kernel_spmd(nc, in_maps, core_ids=list(range(8)))
    out = np.zeros((B, S, D), np.float32)
    for core in range(8):
        b, p = core // 2, core % 2
        out[b, qrows_for(p), :] = res.results[core]["outT"].T
    return out


# revision 31
# speedup vs baseline: 1.7180x; 1.0331x over previous
"""Trainium2 Bass kernel for causal multi-head attention block (v2).

Reference computation (B=4, S=2048, D=1024, H=16, HD=64, fp32):
    qkv = x @ Wqkv + bqkv; split q,k,v; per-head scaled scores;
    causal mask filled with -0.0001 (leaky, NOT -inf); softmax over all
    2048 keys; out = P @ V; out = out @ Wo + bo.

Sharding: 8 cores, core = (batch b = i//2, parity p = i%2). Each core
computes 1024 queries of its batch: query tiles t = 2j+p (j=0..3) of
256 queries -> identical SPMD program, zero cross-core communication.

v2 design vs v1 baseline (602us -> ~430us traced):
  - all inputs bf16 host-side (half DMA, FWL weight loads, 2x DVE).
  - leaky diag mask via one copy_predicated (uint8 predicate, bf16
    w-filled data tile) instead of two full tensor_tensor passes.
  - V bias folded into bo' = bo + bv @ Wo on host (exact).
  - 1/8 score scale folded into the exp activation's scale field.
  - all Q/K/V/out bias-add casts moved from DVE to the ACT engine
    (Identity activation with per-partition AP bias) - ACT is idle
    outside the exp stream; DVE was the attention-phase bottleneck.
  - all weights resident in SBUF up front; K-proj of pair pr+1 emitted
    inside pair pr's attention loop as PE filler (keeps HAM warm);
    out-projection for qtile j emitted inside pr=7's attention.
  - V projection in N=512 matmuls (8 heads/group).
  - attention output kept in SBUF (no DRAM roundtrip).
"""

import math
from contextlib import ExitStack

import numpy as np

import concourse.bass as bass
import concourse.mybir as mybir
import concourse.tile as tile
from concourse import bacc

F32 = mybir.dt.float32
F32R = mybir.dt.float32r
BF16 = mybir.dt.bfloat16
AF = mybir.ActivationFunctionType
ALU = mybir.AluOpType
AX = mybir.AxisListType

B, S, D, H, HD = 4, 2048, 1024, 16, 64
QL, QT, KB, NJ = 1024, 256, 512, 4    # queries/core, qtile, key block, n qtiles
NCH = D // 128                         # contraction chunks
PAIRS = H // 2
W_MASK = math.exp(-1e-4)


def build_program():
    nc = bacc.Bacc(
        "TRN2",
        target_bir_lowering=False,
        debug=False,
        num_devices=8,
    )
    xT = nc.declare_dram_parameter("xT", [D, S], BF16, isOutput=False)
    xqT = nc.declare_dram_parameter("xqT", [D, QL], BF16, isOutput=False)
    wq = nc.declare_dram_parameter("wq", [D, D], BF16, isOutput=False)
    wk = nc.declare_dram_parameter("wk", [D, D], BF16, isOutput=False)
    wv = nc.declare_dram_parameter("wv", [D, D], BF16, isOutput=False)
    wo = nc.declare_dram_parameter("wo", [D, D], BF16, isOutput=False)
    bq2 = nc.declare_dram_parameter("bq2", [128, 8], F32, isOutput=False)
    bk2 = nc.declare_dram_parameter("bk2", [128, 8], F32, isOutput=False)
    bo2 = nc.declare_dram_parameter("bo2", [128, 8], F32, isOutput=False)
    maskw = nc.declare_dram_parameter("maskw", [128, 4 * QT], BF16, isOutput=False)
    maskp = nc.declare_dram_parameter("maskp", [128, 4 * QT], mybir.dt.uint8, isOutput=False)
    onesd = nc.declare_dram_parameter("onesd", [1, 512], F32R, isOutput=False)
    outT = nc.declare_dram_parameter("outT", [D, QL], F32, isOutput=True)

    with tile.TileContext(nc) as tc, ExitStack() as ctx, \
         nc.allow_low_precision(reason="bf16 matmul inputs within rel-err budget"):
        consts = ctx.enter_context(tc.tile_pool(name="consts", bufs=1))
        bq_sb = consts.tile([128, 8], F32)
        nc.sync.dma_start(out=bq_sb, in_=bq2[:])
        bk_sb = consts.tile([128, 8], F32)
        nc.sync.dma_start(out=bk_sb, in_=bk2[:])
        bo_sb = consts.tile([128, 8], F32)
        nc.sync.dma_start(out=bo_sb, in_=bo2[:])
        maskw_sb = consts.tile([128, 4, QT], BF16)
        nc.sync.dma_start(out=maskw_sb, in_=maskw[:].rearrange("p (a b) -> p a b", a=4))
        maskp_sb = consts.tile([128, 4, QT], mybir.dt.uint8)
        nc.sync.dma_start(out=maskp_sb, in_=maskp[:].rearrange("p (a b) -> p a b", a=4))
        ones_sb = consts.tile([1, 512], F32R)
        nc.sync.dma_start(out=ones_sb, in_=onesd[:])

        # ---------- resident activations + weights ----------
        # DMA order: Q-projection inputs first so the PE can start ASAP.
        w_pool = ctx.enter_context(tc.tile_pool(name="wsb", bufs=1))
        wq_sb = w_pool.tile([128, NCH, D], BF16)
        nc.sync.dma_start(out=wq_sb, in_=wq[:].rearrange("(c p) m -> p c m", p=128))
        xt_pool = ctx.enter_context(tc.tile_pool(name="xt", bufs=1))
        xT_sb = xt_pool.tile([128, NCH, S], BF16)
        wv_sb = w_pool.tile([128, NCH, D], BF16)
        wk_sb = w_pool.tile([128, NCH, D], BF16)
        wo_sb = w_pool.tile([128, NCH, 8, 128], BF16)

        xsum_sb = consts.tile([128, NCH, 4], BF16)

        qt_pool = ctx.enter_context(tc.tile_pool(name="qt", bufs=1))
        QT_all = qt_pool.tile([128, PAIRS, QL], BF16)
        v_pool = ctx.enter_context(tc.tile_pool(name="vsb", bufs=1))
        V_sb = v_pool.tile([128, 16, H, 65], BF16)
        nc.gpsimd.memset(V_sb[:, :, :, 64:65], 1.0)
        o_pool = ctx.enter_context(tc.tile_pool(name="osb", bufs=1))
        O_sb = o_pool.tile([128, NCH, QL], BF16)

        # ---------- Q projection (queries for this core only) ----------
        with tc.tile_pool(name="xq", bufs=1) as xq_pool, \
             tc.tile_pool(name="qps", bufs=2, space="PSUM") as qps_pool:
            xqT_sb = xq_pool.tile([128, NCH, QL], BF16)
            for c in range(NCH):
                nc.sync.dma_start(out=xqT_sb[:, c, :], in_=xqT[128 * c:128 * (c + 1), :])
            for c in range(NCH):
                nc.sync.dma_start(out=xT_sb[:, c, :], in_=xT[128 * c:128 * (c + 1), :])
            nc.sync.dma_start(out=wv_sb, in_=wv[:].rearrange("(c p) m -> p c m", p=128))
            nc.sync.dma_start(out=wk_sb, in_=wk[:].rearrange("(c p) m -> p c m", p=128))
            nc.sync.dma_start(
                out=wo_sb, in_=wo[:].rearrange("(c p) (t m) -> p c t m", p=128, m=128)
            )
            # per-512-block column sums of xT, pre-scaled by w (V suffix sums)
            for c in range(NCH):
                nc.vector.tensor_reduce(
                    out=xsum_sb[:, c, :],
                    in_=xT_sb[:, c, :].rearrange("p (b t) -> p b t", b=4),
                    axis=AX.X, op=ALU.add,
                )
            nc.gpsimd.tensor_scalar_mul(
                out=xsum_sb[:].rearrange("p c b -> p (c b)"),
                in0=xsum_sb[:].rearrange("p c b -> p (c b)"),
                scalar1=W_MASK,
            )
            for pr in range(PAIRS):
                for g2 in range(2):
                    ps = qps_pool.tile([128, 512], F32)
                    for c in range(NCH):
                        nc.tensor.matmul(
                            out=ps, lhsT=wq_sb[:, c, 128 * pr:128 * (pr + 1)],
                            rhs=xqT_sb[:, c, 512 * g2:512 * (g2 + 1)],
                            start=(c == 0), stop=(c == NCH - 1),
                        )
                    nc.scalar.activation(
                        out=QT_all[:, pr, 512 * g2:512 * (g2 + 1)], in_=ps,
                        func=AF.Identity, bias=bq_sb[:, pr:pr + 1],
                    )


        # ---------- main loop over head pairs ----------
        kt_pool = ctx.enter_context(tc.tile_pool(name="kt", bufs=2))
        suf_pool = ctx.enter_context(tc.tile_pool(name="suf", bufs=2))

        with tc.tile_pool(name="pps", bufs=2, space="PSUM") as pps_pool, \
             tc.tile_pool(name="sps", bufs=2, space="PSUM") as sps_pool, \
             tc.tile_pool(name="ops", bufs=2, space="PSUM") as ops_pool, \
             tc.tile_pool(name="esb", bufs=4) as e_pool, \
             tc.tile_pool(name="zsb", bufs=2) as z_pool, \
             tc.tile_pool(name="fout", bufs=3) as fo_pool:

            def emit_kproj_kg(pr, KT_sb, kg):
                ps = pps_pool.tile([128, 512], F32, tag="pps")
                for c in range(NCH):
                    nc.tensor.matmul(
                        out=ps, lhsT=wk_sb[:, c, 128 * pr:128 * (pr + 1)],
                        rhs=xT_sb[:, c, 512 * kg:512 * (kg + 1)],
                        start=(c == 0), stop=(c == NCH - 1),
                    )
                nc.scalar.activation(
                    out=KT_sb[:, 512 * kg:512 * (kg + 1)], in_=ps,
                    func=AF.Identity, bias=bk_sb[:, pr:pr + 1],
                )

            def emit_suf(pr):
                # psb[d, b] = w * sum_{tok in block b} V[tok, d]; suffix over b
                psb = pps_pool.tile([128, 4], F32, tag="pps")
                for c in range(NCH):
                    nc.tensor.matmul(
                        out=psb, lhsT=wv_sb[:, c, 128 * pr:128 * (pr + 1)],
                        rhs=xsum_sb[:, c, :],
                        start=(c == 0), stop=(c == NCH - 1),
                    )
                sufT = suf_pool.tile([64, 2, 4], F32)
                for hb in range(2):
                    hs = slice(64 * hb, 64 * hb + 64)
                    nc.vector.memset(sufT[:, hb, 3:4], 0.0)
                    nc.vector.tensor_copy(out=sufT[:, hb, 2:3], in_=psb[hs, 3:4])
                    nc.vector.tensor_add(
                        out=sufT[:, hb, 1:2], in0=psb[hs, 2:3], in1=sufT[:, hb, 2:3])
                    nc.vector.tensor_add(
                        out=sufT[:, hb, 0:1], in0=psb[hs, 1:2], in1=sufT[:, hb, 1:2])
                return sufT

            def emit_vproj(pr2, trange):
                for t in trange:
                    ps = pps_pool.tile([128, 128], F32, tag="pps", name="vp")
                    for c in range(NCH):
                        nc.tensor.matmul(
                            out=ps, lhsT=xT_sb[:, c, 128 * t:128 * (t + 1)],
                            rhs=wv_sb[:, c, 128 * pr2:128 * (pr2 + 1)],
                            start=(c == 0), stop=(c == NCH - 1),
                        )
                    nc.scalar.activation(
                        out=V_sb[:, t, 2 * pr2:2 * pr2 + 2, 0:64],
                        in_=ps.rearrange("p (h d) -> p h d", h=2),
                        func=AF.Identity,
                    )

            def emit_outproj(jj):
                for dt_ in range(8):
                    ps = pps_pool.tile([128, 256], F32, tag="pps", name="fps")
                    for c in range(NCH):
                        nc.tensor.matmul(
                            out=ps, lhsT=wo_sb[:, c, dt_, :],
                            rhs=O_sb[:, c, 256 * jj:256 * (jj + 1)],
                            start=(c == 0), stop=(c == NCH - 1),
                        )
                    fo = fo_pool.tile([128, 256], F32, name="fo")
                    nc.scalar.activation(
                        out=fo, in_=ps, func=AF.Identity, bias=bo_sb[:, dt_:dt_ + 1])
                    nc.sync.dma_start(
                        out=outT[128 * dt_:128 * (dt_ + 1), 256 * jj:256 * (jj + 1)],
                        in_=fo,
                    )

            emit_vproj(0, range(16))
            KT_cur = kt_pool.tile([128, S], BF16)
            for kg in range(4):
                emit_kproj_kg(0, KT_cur, kg)
            suf_cur = emit_suf(0)

            for pr in range(PAIRS):
                if pr + 1 < PAIRS:
                    KT_nxt = kt_pool.tile([128, S], BF16)
                else:
                    KT_nxt = None
                suf_nxt = None
                for j in range(NJ):
                    po = [None, None]
                    e_kb = [None, None]
                    for kb in range(j + 1):
                        diag = kb == j
                        # scores for both heads of the pair, row-tiled to run
                        # concurrently on the two 64-row halves of the PE
                        e_kb2 = [None, None]
                        for hl in range(2):
                            hsl = slice(64 * hl, 64 * (hl + 1))
                            pss = sps_pool.tile([128, 4, 256], F32, tag="sps", name="pss")
                            for s2 in range(4):
                                nc.tensor.matmul(
                                    out=pss[:, s2, :],
                                    lhsT=KT_cur[hsl, 512 * kb + 128 * s2:512 * kb + 128 * (s2 + 1)],
                                    rhs=QT_all[hsl, pr, 256 * j:256 * (j + 1)],
                                    start=True, stop=True,
                                )
                            e_sb = e_pool.tile([128, 4, 256], BF16, tag="e")
                            nc.scalar.activation(
                                out=e_sb, in_=pss, func=AF.Exp, scale=0.125)
                            if diag:
                                nc.vector.copy_predicated(e_sb, maskp_sb, maskw_sb)
                            e_kb2[hl] = e_sb
                        for hl in range(2):
                            if po[hl] is None:
                                po[hl] = ops_pool.tile([65, 512], F32, tag="ops", name="po")
                            for s2 in range(4):
                                nc.tensor.matmul(
                                    out=po[hl][:, 0:256],
                                    lhsT=V_sb[:, 4 * kb + s2, 2 * pr + hl, :],
                                    rhs=e_kb2[hl][:, s2, :],
                                    start=(kb == 0 and s2 == 0),
                                    stop=(kb == j and s2 == 3),
                                    skip_group_check=True,
                                )
                        # PE filler: next pair's K projection
                        if kb == 0 and KT_nxt is not None:
                            emit_kproj_kg(pr + 1, KT_nxt, j)
                        if j == NJ - 1 and kb == 2 and KT_nxt is not None:
                            suf_nxt = emit_suf(pr + 1)
                    for hl in range(2):
                        zf = z_pool.tile([1, 256], F32, tag="zf")
                        nc.vector.tensor_scalar_add(
                            out=zf, in0=po[hl][64:65, 0:256],
                            scalar1=W_MASK * (S - KB * (j + 1)),
                        )
                        zi = z_pool.tile([1, 256], F32, tag="zi")
                        nc.vector.reciprocal_approx_fast(out=zi, in_=zf)
                        zr = z_pool.tile([1, 256], F32R, tag="zr")
                        nc.vector.tensor_copy(out=zr, in_=zi)
                        nc.tensor.matmul(
                            out=po[hl][0:64, 256:512],
                            lhsT=ones_sb[0:1, 0:64], rhs=zr,
                            start=True, stop=True,
                        )
                        nm = z_pool.tile([64, 256], F32, tag="nm")
                        nc.vector.tensor_scalar_add(
                            out=nm, in0=po[hl][0:64, 0:256],
                            scalar1=suf_cur[:, hl, j:j + 1],
                        )
                        nc.vector.tensor_mul(
                            out=O_sb[64 * hl:64 * (hl + 1), pr, 256 * j:256 * (j + 1)],
                            in0=nm, in1=po[hl][0:64, 256:512],
                        )
                    if pr + 1 < PAIRS:
                        emit_vproj(pr + 1, range(4 * j, 4 * j + 4))
                    if pr == PAIRS - 1:
                        emit_outproj(j)
                KT_cur = KT_nxt
                suf_cur = suf_nxt

    nc.compile()
    return nc


def qrows_for(p):
    return np.concatenate(
        [np.arange(QT * (2 * j + p), QT * (2 * j + p) + QT) for j in range(NJ)]
    )


def host_in_maps(x, Wqkv, bqkv, Wo, bo):
    import ml_dtypes
    bf16 = ml_dtypes.bfloat16

    x = np.asarray(x, np.float32)
    Wqkv = np.asarray(Wqkv, np.float32)
    bqkv = np.asarray(bqkv, np.float32)
    Wo = np.asarray(Wo, np.float32)
    bo = np.asarray(bo, np.float32)

    wq_h = np.ascontiguousarray(Wqkv[:, 0:D]).astype(bf16)
    wk_h = np.ascontiguousarray(Wqkv[:, D:2 * D]).astype(bf16)
    wv_h = np.ascontiguousarray(Wqkv[:, 2 * D:3 * D]).astype(bf16)
    wo_h = np.ascontiguousarray(Wo).astype(bf16)

    bq2 = np.ascontiguousarray(bqkv[0:D].reshape(8, 128).T)
    bk2 = np.ascontiguousarray(bqkv[D:2 * D].reshape(8, 128).T)
    # V bias folded into the output-projection bias: out = num/Z + bv -> @Wo
    bo_eff = bo + bqkv[2 * D:3 * D] @ Wo
    bo2 = np.ascontiguousarray(bo_eff.reshape(8, 128).T.astype(np.float32))
    onesd = np.ones((1, 512), np.float32)

    kap = np.arange(128)[:, None]
    r = np.arange(QT)[None, :]
    masks = {}
    for p in range(2):
        mm = np.zeros((128, 4, QT), np.float32)
        for s in range(4):
            mm[:, s, :] = (128 * s + kap <= QT * p + r)
        pred = np.ascontiguousarray(
            (1.0 - mm.reshape(128, 4 * QT)).astype(np.uint8))
        masks[p] = pred
    wdata = np.ascontiguousarray(
        np.full((128, 4 * QT), W_MASK, np.float32).astype(bf16))

    in_maps = []
    for core in range(8):
        b, p = core // 2, core % 2
        in_maps.append({
            "xT": np.ascontiguousarray(x[b].T).astype(bf16),
            "xqT": np.ascontiguousarray(x[b][qrows_for(p)].T).astype(bf16),
            "wq": wq_h,
            "wk": wk_h,
            "wv": wv_h,
            "wo": wo_h,
            "bq2": bq2,
            "bk2": bk2,
            "bo2": bo2,
            "maskw": wdata,
            "maskp": masks[p],
            "onesd": onesd,
        })
    return in_maps


_CACHED = {}


def get_program():
    if "nc" not in _CACHED:
        _CACHED["nc"] = build_program()
    return _CACHED["nc"]


def kernel(x, Wqkv, bqkv, Wo, bo):
    from concourse.bass_utils import run_bass_kernel_spmd

    nc = get_program()
    in_maps = host_in_maps(x, Wqkv, bqkv, Wo, bo)
    res = run_bass_kernel_spmd(nc, in_maps, core_ids=list(range(8)))
    out = np.zeros((B, S, D), np.float32)
    for core in range(8):
        b, p = core // 2, core % 2
        out[b, qrows_for(p), :] = res.results[core]["outT"].T
    return out
